# revision 10
# baseline (speedup 1.0000x reference)
"""Trainium2 Bass kernel for nn_EntropicOTQuantileRegression.

Reference computation (N=1024, M=2048, DX=48, DY=8, H=64, EPS=1e-7):
    hx = X @ W1[:DX]                                  [n, h]
    hu = U @ W1[DX:]                                  [m, h]
    h1 = softplus(hx[:,None,:] + hu[None,:,:] + b1)   [n, m, h]
    h2 = softplus(h1 @ W2 + b2)                       [n, m, h]
    phi = (h2 @ W3)[..., 0] + b3[0]                   [n, m]
    slack = Y @ U.T - phi
    psi = EPS*(logsumexp((slack - rowmax)/EPS, axis=1) - log(M)) + rowmax

Sharding: data-parallel over n. Each of the 8 cores gets 128 rows of X/Y and
replicates U + MLP weights. No collectives.

Design (HW-measured: 1341us staged baseline -> ~250us (Pool removed) ->
~150us (this version); per-iteration cost ~2.2us is engine-balanced:
ScalarE 2.19us (sigmoid), DVE ~2.2us (chains+reduce), PE ~2.3us (matmuls)):
- Partition layout stacks two n-rows (h=64: 128 partitions hold rows i, i+64).
- Layer-1 pre-activation is separable: exp(hx+hu+b1) = exp(hx+b1)*exp(hu), so
  softplus1 = ln(Ex2[:,i]*Eu2 + 1), computed entirely on the DVE as a 2-op
  bf16 "Mitchell" chain (t1 = Ex2*Eu2+1; h1 = (bits(t1)-C)*ln2/128).
- Layer-2 uses softplus(x) = -ln(sigmoid(-x)): ONE ScalarE sigmoid op per
  pre2 half (s2 = sigmoid(-(pre2+b2)), table-exact, bf16 out) plus ONE DVE
  Mitchell op (h2 = (C-bits(s2))*ln2/128 >= 0). This removes the old
  Exp+Ln pair (two full ScalarE passes + chains) from the loop; the loop
  touches only the sigmoid act table -> no table thrash.
- The Pool/GPSIMD engine is NOT used in the loop: HW A/B showed dependent
  Pool ops cost ~1.1ms in wake-up stalls (the old 1.34ms baseline collapsed
  to ~250us just by moving Pool work to the DVE).
- pre2 lives in PSUM as TWO half-width tiles ([128,1024] f32, bufs=2, 4
  banks total) so mm1(i+1, h) only waits on sigmoid(i, h) - PE and ScalarE
  ping-pong halves instead of serializing on one full-width buffer.
- slack = cost - phi is accumulated DIRECTLY in PSUM, quad-packed (4
  iterations per [128, M] tile): each quad's tile is seeded with cost rows by
  a PE matmul (Y-quad stationary, start=True), then the phi matmuls
  accumulate NEGATED W3 on top (start=False). One DVE reduce_max per quad
  pulls the row maxima straight out of PSUM - no staging copies and NO
  in-loop DMAs (the old stage+DMA path serialized the loop: ~110us on HW).
- Epilogue: EPS=1e-7 collapses the f32 logsumexp to a row max exactly, so
  psi = rowmax(cost - phi) - EPS*log(M) - b3, read per-quad from qmax.
- Prologue: constants ride the ACT hardware DMA queue while X/Y/U stream on
  the SP queue (U in one strided DMA); big prologue matmuls are bf16 with
  doubled stationaries to dodge the 4x f32 cost at the cold-PE low pstate.
"""

import math
from contextlib import ExitStack

import numpy as np

import concourse.bass as bass
import concourse.bacc as bacc
import concourse.tile as tile
from concourse import mybir
from concourse.bass_utils import run_bass_kernel_spmd
from concourse.masks import make_identity

# Problem constants (hardcoded; kernel.py must be self-contained).
N, M = 1024, 2048
DX, DY = 48, 8
H = 64
EPS = 1e-7
N_CORES = 8
NLOC = N // N_CORES  # 128 rows per core
F32 = mybir.dt.float32
BF16 = mybir.dt.bfloat16
U16 = mybir.dt.uint16
AF = mybir.ActivationFunctionType
ALU = mybir.AluOpType

# Mitchell bit-trick: for bf16 t > 0, ln(t) ~ (bits_u16(t) - C) * ln2/128,
# since bits(t) = 128*(log2 t + 127 + eps(u)), eps in [0, 0.0861]. C centers
# eps; worst-case h-error ~0.03 which is ~15x under the psi error budget
# (numpy end-to-end: l2 rel 1.5e-3).
LN2 = math.log(2.0)
MITCH_C = 16256 - 6  # 127<<7 minus eps-centering

NITER_OVR = None  # diagnostic: run fewer main-loop iterations

_CACHE = {}


def _patch_act_tables():
    """Make Exp/Ln resolve uniquely to the combined natural_log_exp_and_others
    table so `insert_act_table_loads` hoists ONE load instead of thrashing.

    (Prologue uses Exp; the main loop uses only Sigmoid/Copy, which share the
    sigmoid_and_others table -> 2 table loads total.)
    """
    if getattr(bacc, "_act_tables_patched", False):
        return
    orig = bacc.get_activation_tables
    AFT = mybir.ActivationFunctionType

    def patched(arch):
        tabs = dict(orig(arch))
        combined = "natural_log_exp_and_others"
        if combined in tabs and {AFT.Exp, AFT.Ln} <= tabs[combined]:
            tabs = {
                name: (s if name == combined else s - {AFT.Exp, AFT.Ln})
                for name, s in tabs.items()
            }
        return tabs

    bacc.get_activation_tables = patched
    bacc._act_tables_patched = True


def build_program(repeats=1, loop_n=0, niter=None, **_ignored):
    global NITER_OVR
    if niter is not None:
        NITER_OVR = niter
    _patch_act_tables()
    nc = bacc.Bacc(
        "TRN2",
        target_bir_lowering=False,
        debug=False,
        enable_asserts=False,
        num_devices=N_CORES,
    )

    X = nc.dram_tensor("X", (NLOC, DX), F32, kind="ExternalInput").ap()
    U = nc.dram_tensor("U", (M, DY), F32, kind="ExternalInput").ap()
    Y = nc.dram_tensor("Y", (NLOC, DY), F32, kind="ExternalInput").ap()
    W1 = nc.dram_tensor("W1", (DX + DY, H), F32, kind="ExternalInput").ap()
    b1 = nc.dram_tensor("b1", (H,), F32, kind="ExternalInput").ap()
    W2 = nc.dram_tensor("W2", (H, H), F32, kind="ExternalInput").ap()
    b2 = nc.dram_tensor("b2", (H,), F32, kind="ExternalInput").ap()
    W3 = nc.dram_tensor("W3", (H, 1), F32, kind="ExternalInput").ap()
    b3 = nc.dram_tensor("b3", (1,), F32, kind="ExternalInput").ap()
    out = nc.dram_tensor("out", (NLOC, 1), F32, kind="ExternalOutput").ap()

    with tile.TileContext(nc) as tc:
        if loop_n:
            with tc.For_i(0, loop_n, 1):
                with ExitStack() as ctx:
                    _body(ctx, tc, nc, X, U, Y, W1, b1, W2, b2, W3, b3, out)
        else:
            for _ in range(repeats):
                with ExitStack() as ctx:
                    _body(ctx, tc, nc, X, U, Y, W1, b1, W2, b2, W3, b3, out)

    nc.compile()
    return nc


def _body(ctx, tc, nc, X, U, Y, W1, b1, W2, b2, W3, b3, out):
    NFULL = NLOC // 2  # 64: full iteration count; APs below are NFULL-based
    NITER = NITER_OVR or NFULL  # diagnostic override runs fewer iterations
    HALF = M // 2  # 1024: pre2 PSUM half width (2 banks each)

    const = ctx.enter_context(tc.tile_pool(name="const", bufs=1))
    big = ctx.enter_context(tc.tile_pool(name="big", bufs=1))

    # --- small SBUF constants -------------------------------------------
    # Each dma_start costs ~625ns of queue time, so constants are fused into
    # single DMAs (stride-0 repeat APs for the x2 partition stacking) and
    # routed via the ScalarE hardware DMA queue so X/Y/U can stream on the SP
    # queue in parallel. (gpsimd dma_start is SWDGE: ~1us serial on Pool.)
    ident = const.tile([128, 128], F32)
    make_identity(nc, ident)

    rep2 = lambda t, inner: bass.AP(tensor=t, offset=0, ap=[[0, 2]] + inner)

    W1a = const.tile([DX, H], F32)
    nc.scalar.dma_start(out=W1a, in_=W1[0:DX, :])
    W1b = const.tile([DY, H], F32)
    nc.scalar.dma_start(out=W1b, in_=W1[DX : DX + DY, :])

    # b1/b2 stacked twice on 128 partitions: partition p holds b[p % 64]
    b1s = const.tile([128, 1], F32)
    nc.scalar.dma_start(out=b1s, in_=rep2(b1.tensor, [[1, H]]))
    b2s = const.tile([128, 1], F32)
    nc.scalar.dma_start(out=b2s, in_=rep2(b2.tensor, [[1, H]]))
    b3s = const.tile([128, 1], F32)
    nc.scalar.dma_start(out=b3s, in_=b3.unsqueeze(1).partition_broadcast(128))

    # nb2s = -b2 (bias for the sigmoid: s2 = sigmoid(-pre2 - b2))
    nb2s = const.tile([128, 1], F32)
    nc.vector.tensor_scalar(
        out=nb2s, in0=b2s, scalar1=-1.0, scalar2=0.0, op0=ALU.mult, op1=ALU.add
    )

    # W2 block-diagonal stack [128,128] bf16: [[W2, 0], [0, W2]]
    W2f = const.tile([128, H], F32)
    nc.scalar.dma_start(out=W2f, in_=rep2(W2.tensor, [[H, H], [1, H]]))
    W2s = const.tile([128, 128], BF16)
    nc.vector.memset(W2s, 0.0)
    nc.vector.tensor_copy(W2s[0:H, 0:H], W2f[0:H, :])
    nc.vector.tensor_copy(W2s[H : 2 * H, H : 2 * H], W2f[H : 2 * H, :])

    # NEGATED W3 stack [128, 32] bf16: cols 0/1 = the two -W3 halves, cols
    # 2..31 zero (32-wide so each quad phi matmul writes a full 32-partition
    # col group). Negated because the quad PSUM tile accumulates
    # slack = cost - phi directly.
    W3f = const.tile([128, 1], F32)
    nc.scalar.dma_start(out=W3f, in_=rep2(W3.tensor, [[1, H]]))
    W3s = const.tile([128, 32], BF16)
    nc.vector.memset(W3s, 0.0)
    nc.vector.tensor_scalar(
        out=W3s[0:H, 0:1], in0=W3f[0:H, :], scalar1=-1.0, scalar2=0.0,
        op0=ALU.mult, op1=ALU.add,
    )
    nc.vector.tensor_scalar(
        out=W3s[H : 2 * H, 1:2], in0=W3f[H : 2 * H, :], scalar1=-1.0,
        scalar2=0.0, op0=ALU.mult, op1=ALU.add,
    )

    # bf16 doubled stationaries so the big prologue matmuls run 1 col/cycle
    # (f32 matmuls cost 4x cycles and hit the cold-PE low pstate).
    W1ab = const.tile([DX, H], BF16)
    nc.vector.tensor_copy(W1ab, W1a)
    W1bb2 = const.tile([DY, 128], BF16)  # [W1b | W1b]
    nc.vector.tensor_copy(W1bb2[:, 0:H], W1b)
    nc.vector.tensor_copy(W1bb2[:, H : 2 * H], W1b)

    # --- transposes (PE) -------------------------------------------------
    X_T = const.tile([DX, 128], BF16)  # X^T
    Y_T = const.tile([DY, 128], BF16)  # Y^T
    U_T = const.tile([DY, M], BF16)  # U^T
    with tc.tile_pool(name="psumA", bufs=1, space="PSUM") as psA, tc.tile_pool(
        name="ld", bufs=4
    ) as ld:
        X_sb = ld.tile([128, DX], F32, tag="xy")
        nc.sync.dma_start(out=X_sb, in_=X)
        X_T_ps = psA.tile([DX, 128], F32, tag="xyt")
        nc.tensor.transpose(X_T_ps, X_sb, ident)
        nc.vector.tensor_copy(X_T, X_T_ps)

        # Y rows loaded in interleaved order q = 2i+p <-> n = i + 64p, so that
        # cost rows line up with the phi layout written by the main loop.
        Y_sb = ld.tile([128, DY], F32, tag="xy")
        Y_perm = bass.AP(
            tensor=Y.tensor,
            offset=Y.offset,
            ap=[[DY, NFULL], [NFULL * DY, 2], [1, DY]],
        )
        nc.sync.dma_start(out=Y_sb, in_=Y_perm)
        Y_T_ps = psA.tile([DY, 128], F32, tag="xyt")
        nc.tensor.transpose(Y_T_ps, Y_sb, ident)
        nc.vector.tensor_copy(Y_T, Y_T_ps)

        # U in ONE DMA: partition p, column group k holds U[k*128+p, :]
        U_all = ld.tile([128, (M // 128) * DY], F32, tag="uall")
        U_perm = bass.AP(
            tensor=U.tensor,
            offset=0,
            ap=[[DY, 128], [128 * DY, M // 128], [1, DY]],
        )
        nc.sync.dma_start(out=U_all, in_=U_perm)
        U_T_ps = psA.tile([DY, M], F32)
        for k in range(M // 128):
            nc.tensor.transpose(
                U_T_ps[:, k * 128 : (k + 1) * 128],
                U_all[:, k * DY : (k + 1) * DY],
                ident,
            )
        nc.vector.tensor_copy(U_T, U_T_ps)

    # --- Ex2 = exp(hx+b1) stacked, Eu2 = exp(hu) stacked ----------------
    Ex2 = const.tile([128, NFULL], F32)
    Eu2 = big.tile([128, M], BF16)
    # costS[g]: [DY, 128] stationary whose col 32q+p is Y_T col 2(4g+q)+p
    # (i.e. Y rows for quad g at exactly the partitions the quad's phi
    # matmuls write), other cols zero. Seeding the quad PSUM tile with a
    # costS matmul (start=True) initializes it to cost for those rows.
    costS = const.tile([DY, 16 * 128], BF16)
    nc.vector.memset(costS, 0.0)
    with tc.tile_pool(name="psumB", bufs=1, space="PSUM") as psB, tc.tile_pool(
        name="psumC", bufs=2, space="PSUM"
    ) as psC:
        # hx2[p, i] = (X @ W1a)[i + 64*(p>=64), p%64]; stacked columns.
        hx2_ps = psB.tile([128, NFULL], F32)
        nc.tensor.matmul(hx2_ps[0:H, :], W1ab, X_T[:, 0:NFULL], start=True, stop=True)
        nc.tensor.matmul(
            hx2_ps[H : 2 * H, :],
            W1ab,
            X_T[:, NFULL : 2 * NFULL],
            start=True,
            stop=True,
            tile_position=(0, 64),
        )
        nc.scalar.activation(Ex2, hx2_ps, AF.Exp, bias=b1s, scale=1.0)

        # hu2 = U @ W1b replicated on both partition halves via the doubled
        # stationary [W1b | W1b] (one matmul per chunk instead of two).
        hu2_ps = psB.tile([128, M], F32)
        for j in range(M // 512):
            sl = slice(j * 512, (j + 1) * 512)
            nc.tensor.matmul(hu2_ps[:, sl], W1bb2, U_T[:, sl], start=True, stop=True)
        nc.scalar.activation(Eu2, hu2_ps, AF.Exp, bias=0.0, scale=1.0)

        for g in range(16):
            dst = bass.AP(
                tensor=costS.tensor,
                offset=costS.offset + g * 128,
                ap=[[costS.ap[0][0], DY], [32, 4], [1, 2]],
            )
            nc.vector.tensor_copy(dst, Y_T[:, 8 * g : 8 * g + 8])

    # --- main loop -------------------------------------------------------
    # Quad g's slack rows live at partitions 32q+{0,1}; qmax4[:, 4g+j] holds
    # the row maxima of quad g's j-th 512-col chunk (chunked so the next
    # quad's seed matmul chunk j only waits on reduce chunk j, and so the
    # DVE never takes a full-width reduce burst in one step).
    qmax4 = big.tile([128, 64], F32)
    h1_pool = ctx.enter_context(tc.tile_pool(name="h1", bufs=4))
    t1_pool = ctx.enter_context(tc.tile_pool(name="t1", bufs=3))
    s2_pool = ctx.enter_context(tc.tile_pool(name="s2", bufs=6))
    h2_pool = ctx.enter_context(tc.tile_pool(name="h2", bufs=3))
    pre2_pool = ctx.enter_context(tc.tile_pool(name="pre2", bufs=2, space="PSUM"))
    phi_pool = ctx.enter_context(tc.tile_pool(name="phip", bufs=1, space="PSUM"))

    def emit_ln1(i):
        # softplus1 all-DVE: t1 = Ex2[:,i]*Eu2 + 1 (bf16), h1 = Mitchell(t1)
        h1 = h1_pool.tile([128, M], BF16, tag="h1", name=f"h1_{i}")
        t1 = t1_pool.tile([128, M], BF16, tag="t1", name=f"t1_{i}")
        nc.vector.tensor_scalar(
            out=t1, in0=Eu2, scalar1=Ex2[:, i : i + 1], scalar2=1.0,
            op0=ALU.mult, op1=ALU.add,
        )
        nc.vector.tensor_scalar(
            out=h1, in0=t1.bitcast(U16), scalar1=MITCH_C, scalar2=LN2 / 128.0,
            op0=ALU.subtract, op1=ALU.mult,
        )
        return h1

    def emit_mm1(i, h1):
        # pre2 = W2s.T @ h1 per half (block-diag -> both row-halves at once)
        halves = []
        for h in range(2):
            pre2_ps = pre2_pool.tile(
                [128, HALF], F32, tag="pre2", name=f"pre2_{i}_{h}"
            )
            for j in range(2):
                sl_o = slice(j * 512, (j + 1) * 512)
                sl_i = slice(h * HALF + j * 512, h * HALF + (j + 1) * 512)
                nc.tensor.matmul(
                    pre2_ps[:, sl_o], W2s, h1[:, sl_i], start=True, stop=True
                )
            halves.append(pre2_ps)
        return halves

    def emit_sig(i, pre2_halves):
        # softplus2 part 1: s2 = sigmoid(-(pre2 + b2)) on ScalarE (bf16 out)
        s2h = []
        for h in range(2):
            s2 = s2_pool.tile([128, HALF], BF16, tag="s2", name=f"s2_{i}_{h}")
            nc.scalar.activation(
                s2, pre2_halves[h], AF.Sigmoid, bias=nb2s, scale=-1.0
            )
            s2h.append(s2)
        return s2h

    def emit_h2_phi(k, s2h, phi_quad):
        # softplus2 part 2: h2 = -ln(s2) via one DVE Mitchell op per half,
        # then the phi matmul quad piece for iteration k.
        h2 = h2_pool.tile([128, M], BF16, tag="h2", name=f"h2_{k}")
        for h in range(2):
            nc.vector.tensor_scalar(
                out=h2[:, h * HALF : (h + 1) * HALF], in0=s2h[h].bitcast(U16),
                scalar1=MITCH_C, scalar2=-LN2 / 128.0,
                op0=ALU.subtract, op1=ALU.mult,
            )
        q = k % 4
        for j in range(M // 512):
            sl = slice(j * 512, (j + 1) * 512)
            nc.tensor.matmul(
                phi_quad[32 * q : 32 * q + 32, sl], W3s, h2[:, sl],
                start=False, stop=True,
                tile_position=(0, 32 * q) if q else None,
                skip_group_check=True,
            )

    def emit_quad_seed(g):
        # allocate quad g's PSUM tile and seed it with cost = Y @ U.T for the
        # 8 rows this quad owns (start=True resets the banks; the phi
        # matmuls then accumulate -W3 . h2 on top -> slack in PSUM).
        phi_quad = phi_pool.tile([128, M], F32, tag="phi", name=f"phiq_{g}")
        for j in range(M // 512):
            sl = slice(j * 512, (j + 1) * 512)
            nc.tensor.matmul(
                phi_quad[:, sl], costS[:, g * 128 : (g + 1) * 128],
                U_T[:, sl], start=True, stop=False, skip_group_check=True,
            )
        return phi_quad

    def emit_reduce(g, phi_quad, j):
        # slack row maxima for quad g's j-th 512-col chunk, out of PSUM
        sl = slice(j * 512, (j + 1) * 512)
        nc.vector.reduce_max(
            out=qmax4[:, 4 * g + j : 4 * g + j + 1], in_=phi_quad[:, sl],
            axis=mybir.AxisListType.X,
        )

    # Software pipeline: Ln1 two iterations ahead (DVE), mm1 one ahead (PE),
    # sigmoid at i (ScalarE), h2+phi one behind (DVE+PE). pre2 half-buffers
    # let mm1(i+1, h) start as soon as sigmoid(i, h) drains its half.
    h1_ahead = {0: emit_ln1(0), 1: emit_ln1(1)}
    pre2_ahead = {0: emit_mm1(0, h1_ahead.pop(0))}
    s2_live = {}
    phi_quads = {}

    def step(i):
        # pending chunk-reduces of the JUST-CLOSED quad go first on the DVE
        # (the seed matmul chunks emitted below wait on them chunk-by-chunk)
        k = i - 1
        if k >= 0 and k % 4 == 0 and k // 4 > 0:
            gprev = k // 4 - 1
            emit_reduce(gprev, phi_quads[gprev], 2)
            emit_reduce(gprev, phi_quads[gprev], 3)

        s2_live[i] = emit_sig(i, pre2_ahead.pop(i))

        # hoist next iteration's mm1 so PE runs it during the sigmoid window
        if i + 1 < NITER:
            pre2_ahead[i + 1] = emit_mm1(i + 1, h1_ahead.pop(i + 1))

        # softplus2 part 2 + phi for the PREVIOUS iteration
        if k >= 0:
            if k % 4 == 0:
                phi_quads[k // 4] = emit_quad_seed(k // 4)
            emit_h2_phi(k, s2_live.pop(k), phi_quads[k // 4])

        if i + 2 < NITER:
            h1_ahead[i + 2] = emit_ln1(i + 2)

        # first two chunk-reduces at quad close, AFTER ln1 so the h1
        # pipeline never stalls behind them
        if k >= 0 and k % 4 == 3:
            g = k // 4
            emit_reduce(g, phi_quads[g], 0)
            emit_reduce(g, phi_quads[g], 1)

    for i in range(NITER):
        step(i)

    # drain the last delayed iteration
    k = NITER - 1
    if k % 4 == 0:
        phi_quads[k // 4] = emit_quad_seed(k // 4)
    emit_h2_phi(k, s2_live.pop(k), phi_quads[k // 4])
    g = k // 4
    for j in range(4):
        emit_reduce(g, phi_quads[g], j)

    # --- final: psi = qmax - EPS*log(M) - b3 -----------------------------
    # With EPS=1e-7 the f32 logsumexp collapses to the row max: the exp of
    # the second-best gap underflows, so the correction is exactly -EPS*log(M)
    # (bounded by EPS*log(M) ~ 7.6e-7 in all cases - far below tolerance).
    fin = ctx.enter_context(tc.tile_pool(name="fin", bufs=1))
    base = fin.tile([128, 1], F32)
    # base = -b3 - EPS*log(M)
    nc.vector.tensor_scalar(
        out=base, in0=b3s, scalar1=-1.0, scalar2=-EPS * math.log(M),
        op0=ALU.mult, op1=ALU.add,
    )
    # combine the 4 chunk-maxima per quad, then add base
    qmaxq = fin.tile([128, 16], F32)
    qin = bass.AP(
        tensor=qmax4.tensor, offset=qmax4.offset,
        ap=[qmax4.ap[0], [4, 16], [1, 4]],
    )
    nc.vector.reduce_max(out=qmaxq, in_=qin, axis=mybir.AxisListType.X)
    qmaxc = fin.tile([128, 16], F32)
    nc.vector.tensor_scalar(
        out=qmaxc, in0=qmaxq, scalar1=base, scalar2=0.0,
        op0=ALU.add, op1=ALU.add,
    )
    # Row n = (4g+q) + 64p lives at qmaxc[32q+p, g]: one DMA per q writes
    # out[n] for (p, g) via a strided DRAM AP.
    for q in range(4):
        out_ap = bass.AP(
            tensor=out.tensor, offset=out.offset + q,
            ap=[[NFULL, 2], [4, 16]],
        )
        nc.sync.dma_start(out=out_ap, in_=qmaxc[32 * q : 32 * q + 2, :])


def kernel(**inputs):
    if "nc" not in _CACHE:
        _CACHE["nc"] = build_program()
    nc = _CACHE["nc"]

    f32 = lambda a: np.ascontiguousarray(np.asarray(a, dtype=np.float32))
    X = f32(inputs["X"])
    U = f32(inputs["U"])
    Y = f32(inputs["Y"])
    shared = dict(
        U=U,
        W1=f32(inputs["W1"]),
        b1=f32(inputs["b1"]),
        W2=f32(inputs["W2"]),
        b2=f32(inputs["b2"]),
        W3=f32(inputs["W3"]),
        b3=f32(inputs["b3"]),
    )
    in_maps = [
        dict(
            X=X[c * NLOC : (c + 1) * NLOC],
            Y=Y[c * NLOC : (c + 1) * NLOC],
            **shared,
        )
        for c in range(N_CORES)
    ]
    res = run_bass_kernel_spmd(nc, in_maps, core_ids=list(range(N_CORES)))
    return np.concatenate([res.results[c]["out"] for c in range(N_CORES)], axis=0)


if __name__ == "__main__":
    rng = np.random.default_rng(0)
    ins = {
        "X": rng.standard_normal((N, DX), dtype=np.float32),
        "U": rng.standard_normal((M, DY), dtype=np.float32),
        "Y": rng.standard_normal((N, DY), dtype=np.float32),
        "W1": (rng.standard_normal((DX + DY, H)) * 0.1).astype(np.float32),
        "b1": np.zeros(H, np.float32),
        "W2": (rng.standard_normal((H, H)) * 0.1).astype(np.float32),
        "b2": np.zeros(H, np.float32),
        "W3": (rng.standard_normal((H, 1)) * 0.1).astype(np.float32),
        "b3": np.zeros(1, np.float32),
    }
    out = kernel(**ins)
    print(out.shape, out[:4, 0])


# revision 17
# speedup vs baseline: 1.2204x; 1.2204x over previous
"""Trainium2 Bass kernel for nn_EntropicOTQuantileRegression.

Reference computation (N=1024, M=2048, DX=48, DY=8, H=64, EPS=1e-7):
    hx = X @ W1[:DX]                                  [n, h]
    hu = U @ W1[DX:]                                  [m, h]
    h1 = softplus(hx[:,None,:] + hu[None,:,:] + b1)   [n, m, h]
    h2 = softplus(h1 @ W2 + b2)                       [n, m, h]
    phi = (h2 @ W3)[..., 0] + b3[0]                   [n, m]
    slack = Y @ U.T - phi
    psi = EPS*(logsumexp((slack - rowmax)/EPS, axis=1) - log(M)) + rowmax

Sharding: data-parallel over n. Each of the 8 cores gets 128 rows of X/Y and
replicates U + MLP weights. No collectives.

Design (HW-measured: 1341us staged baseline -> ~250us (Pool removed) ->
~150us (this version); per-iteration cost ~2.2us is engine-balanced:
ScalarE 2.19us (sigmoid), DVE ~2.2us (chains+reduce), PE ~2.3us (matmuls)):
- Partition layout stacks two n-rows (h=64: 128 partitions hold rows i, i+64).
- Layer-1 pre-activation is separable: exp(hx+hu+b1) = exp(hx+b1)*exp(hu), so
  softplus1 = ln(Ex2[:,i]*Eu2 + 1), computed entirely on the DVE as a 2-op
  bf16 "Mitchell" chain (t1 = Ex2*Eu2+1; h1 = (bits(t1)-C)*ln2/128).
- Layer-2 uses softplus(x) = -ln(sigmoid(-x)): ONE ScalarE sigmoid op per
  pre2 half (s2 = sigmoid(-(pre2+b2)), table-exact, bf16 out) plus ONE DVE
  Mitchell op (h2 = (C-bits(s2))*ln2/128 >= 0). This removes the old
  Exp+Ln pair (two full ScalarE passes + chains) from the loop; the loop
  touches only the sigmoid act table -> no table thrash.
- The Pool/GPSIMD engine is NOT used in the loop: HW A/B showed dependent
  Pool ops cost ~1.1ms in wake-up stalls (the old 1.34ms baseline collapsed
  to ~250us just by moving Pool work to the DVE).
- pre2 lives in PSUM as TWO half-width tiles ([128,1024] f32, bufs=2, 4
  banks total) so mm1(i+1, h) only waits on sigmoid(i, h) - PE and ScalarE
  ping-pong halves instead of serializing on one full-width buffer.
- slack = cost - phi is accumulated DIRECTLY in PSUM, quad-packed (4
  iterations per [128, M] tile): each quad's tile is seeded with cost rows by
  a PE matmul (Y-quad stationary, start=True), then the phi matmuls
  accumulate NEGATED W3 on top (start=False). One DVE reduce_max per quad
  pulls the row maxima straight out of PSUM - no staging copies and NO
  in-loop DMAs (the old stage+DMA path serialized the loop: ~110us on HW).
- Epilogue: EPS=1e-7 collapses the f32 logsumexp to a row max exactly, so
  psi = rowmax(cost - phi) - EPS*log(M) - b3, read per-quad from qmax.
- Prologue: constants ride the ACT hardware DMA queue while X/Y/U stream on
  the SP queue (U in one strided DMA); big prologue matmuls are bf16 with
  doubled stationaries to dodge the 4x f32 cost at the cold-PE low pstate.
"""

import math
from contextlib import ExitStack

import numpy as np

import concourse.bass as bass
import concourse.bacc as bacc
import concourse.tile as tile
from concourse import mybir
from concourse.bass_utils import run_bass_kernel_spmd
from concourse.masks import make_identity

# Problem constants (hardcoded; kernel.py must be self-contained).
N, M = 1024, 2048
DX, DY = 48, 8
H = 64
EPS = 1e-7
N_CORES = 8
NLOC = N // N_CORES  # 128 rows per core
F32 = mybir.dt.float32
BF16 = mybir.dt.bfloat16
U16 = mybir.dt.uint16
AF = mybir.ActivationFunctionType
ALU = mybir.AluOpType

# Mitchell bit-trick: for bf16 t > 0, ln(t) ~ (bits_u16(t) - C) * ln2/128,
# since bits(t) = 128*(log2 t + 127 + eps(u)), eps in [0, 0.0861]. C centers
# eps; worst-case h-error ~0.03 which is ~15x under the psi error budget
# (numpy end-to-end: l2 rel 1.5e-3).
LN2 = math.log(2.0)
MITCH_C = 16256 - 6  # 127<<7 minus eps-centering

NITER_OVR = None  # diagnostic: run fewer main-loop iterations
# t1 head: columns [0, TSPLIT) of t1 = Ex2*Eu2 + 1 are computed on ScalarE
# via AF.Copy (scale=Ex2 AP, bias=1; Copy shares the sigmoid act table).
# Sim-tested: 0 is best (the Sc queue is the critical cycle; extra Sc ops
# delay the next sigmoid even though the DVE has the higher busy%).
TSPLIT = 0
MMW = 512  # mm1 matmul chunk width (512 = PSUM bank limit per matmul)
# REDSC: quad slack readout mode. 0 = DVE reduce_max straight from PSUM.
# 4 = ScalarE stages the quad to SBUF f32 (AF.Copy, in its quad-boundary
# idle window; releases the phi PSUM banks for the next seed) and the DVE
# reduces from SBUF f32 (0.55ns/col vs 1.27 from PSUM).
REDSC = 0

_CACHE = {}


def _patch_act_tables():
    """Make Exp/Ln resolve uniquely to the combined natural_log_exp_and_others
    table so `insert_act_table_loads` hoists ONE load instead of thrashing.

    (Prologue uses Exp; the main loop uses only Sigmoid/Copy, which share the
    sigmoid_and_others table -> 2 table loads total.)
    """
    if getattr(bacc, "_act_tables_patched", False):
        return
    orig = bacc.get_activation_tables
    AFT = mybir.ActivationFunctionType

    def patched(arch):
        tabs = dict(orig(arch))
        combined = "natural_log_exp_and_others"
        if combined in tabs and {AFT.Exp, AFT.Ln} <= tabs[combined]:
            tabs = {
                name: (s if name == combined else s - {AFT.Exp, AFT.Ln})
                for name, s in tabs.items()
            }
        return tabs

    bacc.get_activation_tables = patched
    bacc._act_tables_patched = True


def build_program(repeats=1, loop_n=0, niter=None, tsplit=None, mmw=None,
                  redsc=None, **_ignored):
    global NITER_OVR, TSPLIT, MMW, REDSC
    if niter is not None:
        NITER_OVR = niter
    if tsplit is not None:
        TSPLIT = tsplit
    if mmw is not None:
        MMW = mmw
    if redsc is not None:
        REDSC = redsc
    _patch_act_tables()
    nc = bacc.Bacc(
        "TRN2",
        target_bir_lowering=False,
        debug=False,
        enable_asserts=False,
        num_devices=N_CORES,
    )

    X = nc.dram_tensor("X", (NLOC, DX), F32, kind="ExternalInput").ap()
    U = nc.dram_tensor("U", (M, DY), F32, kind="ExternalInput").ap()
    Y = nc.dram_tensor("Y", (NLOC, DY), F32, kind="ExternalInput").ap()
    W1 = nc.dram_tensor("W1", (DX + DY, H), F32, kind="ExternalInput").ap()
    b1 = nc.dram_tensor("b1", (H,), F32, kind="ExternalInput").ap()
    W2 = nc.dram_tensor("W2", (H, H), F32, kind="ExternalInput").ap()
    b2 = nc.dram_tensor("b2", (H,), F32, kind="ExternalInput").ap()
    W3 = nc.dram_tensor("W3", (H, 1), F32, kind="ExternalInput").ap()
    b3 = nc.dram_tensor("b3", (1,), F32, kind="ExternalInput").ap()
    out = nc.dram_tensor("out", (NLOC, 1), F32, kind="ExternalOutput").ap()

    with tile.TileContext(nc) as tc:
        if loop_n:
            with tc.For_i(0, loop_n, 1):
                with ExitStack() as ctx:
                    _body(ctx, tc, nc, X, U, Y, W1, b1, W2, b2, W3, b3, out)
        else:
            for _ in range(repeats):
                with ExitStack() as ctx:
                    _body(ctx, tc, nc, X, U, Y, W1, b1, W2, b2, W3, b3, out)

    nc.compile()
    return nc


def _body(ctx, tc, nc, X, U, Y, W1, b1, W2, b2, W3, b3, out):
    NFULL = NLOC // 2  # 64: full iteration count; APs below are NFULL-based
    NITER = NITER_OVR or NFULL  # diagnostic override runs fewer iterations
    HALF = M // 2  # 1024: pre2 PSUM half width (2 banks each)

    const = ctx.enter_context(tc.tile_pool(name="const", bufs=1))
    big = ctx.enter_context(tc.tile_pool(name="big", bufs=1))

    # --- small SBUF constants -------------------------------------------
    # Each dma_start costs ~625ns of queue time, so constants are fused into
    # single DMAs (stride-0 repeat APs for the x2 partition stacking) and
    # routed via the ScalarE hardware DMA queue so X/Y/U can stream on the SP
    # queue in parallel. (gpsimd dma_start is SWDGE: ~1us serial on Pool.)
    ident = const.tile([128, 128], F32)
    make_identity(nc, ident)

    rep2 = lambda t, inner: bass.AP(tensor=t, offset=0, ap=[[0, 2]] + inner)

    W1a = const.tile([DX, H], F32)
    nc.scalar.dma_start(out=W1a, in_=W1[0:DX, :])
    W1b = const.tile([DY, H], F32)
    nc.scalar.dma_start(out=W1b, in_=W1[DX : DX + DY, :])

    # b1/b2 stacked twice on 128 partitions: partition p holds b[p % 64]
    b1s = const.tile([128, 1], F32)
    nc.scalar.dma_start(out=b1s, in_=rep2(b1.tensor, [[1, H]]))
    b2s = const.tile([128, 1], F32)
    nc.scalar.dma_start(out=b2s, in_=rep2(b2.tensor, [[1, H]]))
    b3s = const.tile([128, 1], F32)
    nc.scalar.dma_start(out=b3s, in_=b3.unsqueeze(1).partition_broadcast(128))

    # nb2s = -b2 (bias for the sigmoid: s2 = sigmoid(-pre2 - b2))
    nb2s = const.tile([128, 1], F32)
    nc.vector.tensor_scalar(
        out=nb2s, in0=b2s, scalar1=-1.0, scalar2=0.0, op0=ALU.mult, op1=ALU.add
    )

    # W2 block-diagonal stack [128,128] bf16: [[W2, 0], [0, W2]]
    W2f = const.tile([128, H], F32)
    nc.scalar.dma_start(out=W2f, in_=rep2(W2.tensor, [[H, H], [1, H]]))
    W2s = const.tile([128, 128], BF16)
    nc.vector.memset(W2s, 0.0)
    nc.vector.tensor_copy(W2s[0:H, 0:H], W2f[0:H, :])
    nc.vector.tensor_copy(W2s[H : 2 * H, H : 2 * H], W2f[H : 2 * H, :])

    # NEGATED W3 stack [128, 32] bf16: cols 0/1 = the two -W3 halves, cols
    # 2..31 zero (32-wide so each quad phi matmul writes a full 32-partition
    # col group). Negated because the quad PSUM tile accumulates
    # slack = cost - phi directly.
    W3f = const.tile([128, 1], F32)
    nc.scalar.dma_start(out=W3f, in_=rep2(W3.tensor, [[1, H]]))
    W3s = const.tile([128, 32], BF16)
    nc.vector.memset(W3s, 0.0)
    nc.vector.tensor_scalar(
        out=W3s[0:H, 0:1], in0=W3f[0:H, :], scalar1=-1.0, scalar2=0.0,
        op0=ALU.mult, op1=ALU.add,
    )
    nc.vector.tensor_scalar(
        out=W3s[H : 2 * H, 1:2], in0=W3f[H : 2 * H, :], scalar1=-1.0,
        scalar2=0.0, op0=ALU.mult, op1=ALU.add,
    )

    # bf16 doubled stationaries so the big prologue matmuls run 1 col/cycle
    # (f32 matmuls cost 4x cycles and hit the cold-PE low pstate).
    W1ab = const.tile([DX, H], BF16)
    nc.vector.tensor_copy(W1ab, W1a)
    W1bb2 = const.tile([DY, 128], BF16)  # [W1b | W1b]
    nc.vector.tensor_copy(W1bb2[:, 0:H], W1b)
    nc.vector.tensor_copy(W1bb2[:, H : 2 * H], W1b)

    # --- transposes (PE) -------------------------------------------------
    X_T = const.tile([DX, 128], BF16)  # X^T
    Y_T = const.tile([DY, 128], BF16)  # Y^T
    U_T = const.tile([DY, M], BF16)  # U^T
    with tc.tile_pool(name="psumA", bufs=1, space="PSUM") as psA, tc.tile_pool(
        name="ld", bufs=4
    ) as ld:
        X_sb = ld.tile([128, DX], F32, tag="xy")
        nc.sync.dma_start(out=X_sb, in_=X)
        X_T_ps = psA.tile([DX, 128], F32, tag="xyt")
        nc.tensor.transpose(X_T_ps, X_sb, ident)
        nc.vector.tensor_copy(X_T, X_T_ps)

        # Y rows loaded in interleaved order q = 2i+p <-> n = i + 64p, so that
        # cost rows line up with the phi layout written by the main loop.
        Y_sb = ld.tile([128, DY], F32, tag="xy")
        Y_perm = bass.AP(
            tensor=Y.tensor,
            offset=Y.offset,
            ap=[[DY, NFULL], [NFULL * DY, 2], [1, DY]],
        )
        nc.sync.dma_start(out=Y_sb, in_=Y_perm)
        Y_T_ps = psA.tile([DY, 128], F32, tag="xyt")
        nc.tensor.transpose(Y_T_ps, Y_sb, ident)
        nc.vector.tensor_copy(Y_T, Y_T_ps)

        # U in ONE DMA: partition p, column group k holds U[k*128+p, :]
        U_all = ld.tile([128, (M // 128) * DY], F32, tag="uall")
        U_perm = bass.AP(
            tensor=U.tensor,
            offset=0,
            ap=[[DY, 128], [128 * DY, M // 128], [1, DY]],
        )
        nc.sync.dma_start(out=U_all, in_=U_perm)
        U_T_ps = psA.tile([DY, M], F32)
        for k in range(M // 128):
            nc.tensor.transpose(
                U_T_ps[:, k * 128 : (k + 1) * 128],
                U_all[:, k * DY : (k + 1) * DY],
                ident,
            )
        nc.vector.tensor_copy(U_T, U_T_ps)

    # --- Ex2 = exp(hx+b1) stacked, Eu2 = exp(hu) stacked ----------------
    Ex2 = const.tile([128, NFULL], F32)
    Eu2 = big.tile([128, M], BF16)
    # costS[g]: [DY, 128] stationary whose col 32q+p is Y_T col 2(4g+q)+p
    # (i.e. Y rows for quad g at exactly the partitions the quad's phi
    # matmuls write), other cols zero. Seeding the quad PSUM tile with a
    # costS matmul (start=True) initializes it to cost for those rows.
    costS = const.tile([DY, 16 * 128], BF16)
    nc.vector.memset(costS, 0.0)
    with tc.tile_pool(name="psumB", bufs=1, space="PSUM") as psB, tc.tile_pool(
        name="psumC", bufs=2, space="PSUM"
    ) as psC:
        # hx2[p, i] = (X @ W1a)[i + 64*(p>=64), p%64]; stacked columns.
        hx2_ps = psB.tile([128, NFULL], F32)
        nc.tensor.matmul(hx2_ps[0:H, :], W1ab, X_T[:, 0:NFULL], start=True, stop=True)
        nc.tensor.matmul(
            hx2_ps[H : 2 * H, :],
            W1ab,
            X_T[:, NFULL : 2 * NFULL],
            start=True,
            stop=True,
            tile_position=(0, 64),
        )
        nc.scalar.activation(Ex2, hx2_ps, AF.Exp, bias=b1s, scale=1.0)

        # hu2 = U @ W1b replicated on both partition halves via the doubled
        # stationary [W1b | W1b] (one matmul per chunk instead of two).
        hu2_ps = psB.tile([128, M], F32)
        for j in range(M // 512):
            sl = slice(j * 512, (j + 1) * 512)
            nc.tensor.matmul(hu2_ps[:, sl], W1bb2, U_T[:, sl], start=True, stop=True)
        nc.scalar.activation(Eu2, hu2_ps, AF.Exp, bias=0.0, scale=1.0)

        for g in range(16):
            dst = bass.AP(
                tensor=costS.tensor,
                offset=costS.offset + g * 128,
                ap=[[costS.ap[0][0], DY], [32, 4], [1, 2]],
            )
            nc.vector.tensor_copy(dst, Y_T[:, 8 * g : 8 * g + 8])

    # --- main loop -------------------------------------------------------
    # Quad g's slack rows live at partitions 32q+{0,1}; qmax4[:, 4g+j] holds
    # the row maxima of quad g's j-th 512-col chunk (chunked so the next
    # quad's seed matmul chunk j only waits on reduce chunk j, and so the
    # DVE never takes a full-width reduce burst in one step).
    qmax4 = big.tile([128, 64], F32)
    h1_pool = ctx.enter_context(tc.tile_pool(name="h1", bufs=4))
    t1_pool = ctx.enter_context(tc.tile_pool(name="t1", bufs=3))
    s2_pool = ctx.enter_context(tc.tile_pool(name="s2", bufs=6))
    stage_pool = ctx.enter_context(tc.tile_pool(name="stageq", bufs=2))
    h2_pool = ctx.enter_context(tc.tile_pool(name="h2", bufs=3))
    pre2_pool = ctx.enter_context(tc.tile_pool(name="pre2", bufs=2, space="PSUM"))
    phi_pool = ctx.enter_context(tc.tile_pool(name="phip", bufs=1, space="PSUM"))

    def emit_ln1(i):
        # softplus1: t1 = Ex2[:,i]*Eu2 + 1 (bf16; head cols on ScalarE via
        # AF.Copy with scale AP, rest on DVE), h1 = Mitchell(t1) on DVE
        h1 = h1_pool.tile([128, M], BF16, tag="h1", name=f"h1_{i}")
        t1 = t1_pool.tile([128, M], BF16, tag="t1", name=f"t1_{i}")
        if TSPLIT:
            nc.scalar.activation(
                t1[:, 0:TSPLIT], Eu2[:, 0:TSPLIT], AF.Copy,
                bias=1.0, scale=Ex2[:, i : i + 1],
            )
        nc.vector.tensor_scalar(
            out=t1[:, TSPLIT:M], in0=Eu2[:, TSPLIT:M],
            scalar1=Ex2[:, i : i + 1], scalar2=1.0,
            op0=ALU.mult, op1=ALU.add,
        )
        nc.vector.tensor_scalar(
            out=h1, in0=t1.bitcast(U16), scalar1=MITCH_C, scalar2=LN2 / 128.0,
            op0=ALU.subtract, op1=ALU.mult,
        )
        return h1

    def emit_mm1(i, h1):
        # pre2 = W2s.T @ h1 per half (block-diag -> both row-halves at once)
        halves = []
        for h in range(2):
            pre2_ps = pre2_pool.tile(
                [128, HALF], F32, tag="pre2", name=f"pre2_{i}_{h}"
            )
            for j in range(HALF // MMW):
                sl_o = slice(j * MMW, (j + 1) * MMW)
                sl_i = slice(h * HALF + j * MMW, h * HALF + (j + 1) * MMW)
                nc.tensor.matmul(
                    pre2_ps[:, sl_o], W2s, h1[:, sl_i], start=True, stop=True
                )
            halves.append(pre2_ps)
        return halves

    def emit_sig(i, pre2_halves):
        # softplus2 part 1: s2 = sigmoid(-(pre2 + b2)) on ScalarE (bf16 out)
        s2h = []
        for h in range(2):
            s2 = s2_pool.tile([128, HALF], BF16, tag="s2", name=f"s2_{i}_{h}")
            nc.scalar.activation(
                s2, pre2_halves[h], AF.Sigmoid, bias=nb2s, scale=-1.0
            )
            s2h.append(s2)
        return s2h

    def emit_h2_phi(k, s2h, phi_quad):
        # softplus2 part 2: h2 = -ln(s2) via one DVE Mitchell op per half,
        # then the phi matmul quad piece for iteration k.
        h2 = h2_pool.tile([128, M], BF16, tag="h2", name=f"h2_{k}")
        for h in range(2):
            nc.vector.tensor_scalar(
                out=h2[:, h * HALF : (h + 1) * HALF], in0=s2h[h].bitcast(U16),
                scalar1=MITCH_C, scalar2=-LN2 / 128.0,
                op0=ALU.subtract, op1=ALU.mult,
            )
        q = k % 4
        for j in range(M // 512):
            sl = slice(j * 512, (j + 1) * 512)
            nc.tensor.matmul(
                phi_quad[32 * q : 32 * q + 32, sl], W3s, h2[:, sl],
                start=False, stop=True,
                tile_position=(0, 32 * q) if q else None,
                skip_group_check=True,
            )

    def emit_quad_seed(g):
        # allocate quad g's PSUM tile and seed it with cost = Y @ U.T for the
        # 8 rows this quad owns (start=True resets the banks; the phi
        # matmuls then accumulate -W3 . h2 on top -> slack in PSUM).
        phi_quad = phi_pool.tile([128, M], F32, tag="phi", name=f"phiq_{g}")
        for j in range(M // 512):
            sl = slice(j * 512, (j + 1) * 512)
            nc.tensor.matmul(
                phi_quad[:, sl], costS[:, g * 128 : (g + 1) * 128],
                U_T[:, sl], start=True, stop=False, skip_group_check=True,
            )
        return phi_quad

    stage_live = {}

    def emit_stage(g, phi_quad):
        # ScalarE copies the closed quad PSUM->SBUF f32 in REDSC 512-col
        # chunks (AF.Copy shares the sigmoid table), releasing the phi banks
        # for the next seed without the DVE in the loop.
        stq = stage_pool.tile([128, 512 * REDSC], F32, tag="stq", name=f"stq_{g}")
        for j in range(REDSC):
            sl = slice(j * 512, (j + 1) * 512)
            nc.scalar.activation(
                stq[:, sl], phi_quad[:, sl], AF.Copy, bias=0.0, scale=1.0
            )
        stage_live[g] = stq

    def emit_reduce(g, phi_quad, j):
        # slack row maxima for quad g's j-th 512-col chunk
        sl = slice(j * 512, (j + 1) * 512)
        if j < REDSC:
            nc.vector.reduce_max(
                out=qmax4[:, 4 * g + j : 4 * g + j + 1],
                in_=stage_live[g][:, sl], axis=mybir.AxisListType.X,
            )
        else:
            nc.vector.reduce_max(
                out=qmax4[:, 4 * g + j : 4 * g + j + 1], in_=phi_quad[:, sl],
                axis=mybir.AxisListType.X,
            )

    # Software pipeline: Ln1 two iterations ahead (DVE), mm1 one ahead (PE),
    # sigmoid at i (ScalarE), h2+phi one behind (DVE+PE). pre2 half-buffers
    # let mm1(i+1, h) start as soon as sigmoid(i, h) drains its half.
    h1_ahead = {0: emit_ln1(0), 1: emit_ln1(1)}
    pre2_ahead = {0: emit_mm1(0, h1_ahead.pop(0))}
    s2_live = {}
    phi_quads = {}

    def step(i):
        # pending chunk-reduces of the JUST-CLOSED quad go first on the DVE
        # (the seed matmul chunks emitted below wait on them chunk-by-chunk)
        k = i - 1
        if k >= 0 and k % 4 == 0 and k // 4 > 0:
            gprev = k // 4 - 1
            emit_reduce(gprev, phi_quads[gprev], 2)
            emit_reduce(gprev, phi_quads[gprev], 3)

        s2_live[i] = emit_sig(i, pre2_ahead.pop(i))

        # hoist next iteration's mm1 so PE runs it during the sigmoid window
        if i + 1 < NITER:
            pre2_ahead[i + 1] = emit_mm1(i + 1, h1_ahead.pop(i + 1))

        # softplus2 part 2 + phi for the PREVIOUS iteration
        if k >= 0:
            if k % 4 == 0:
                phi_quads[k // 4] = emit_quad_seed(k // 4)
            emit_h2_phi(k, s2_live.pop(k), phi_quads[k // 4])

        if i + 2 < NITER:
            h1_ahead[i + 2] = emit_ln1(i + 2)

        # at quad close: ScalarE stage (frees the PSUM banks); the first two
        # DVE chunk-reduces stay here only for unstaged chunks
        if k >= 0 and k % 4 == 3:
            g = k // 4
            if REDSC:
                emit_stage(g, phi_quads[g])
            emit_reduce(g, phi_quads[g], 0)
            emit_reduce(g, phi_quads[g], 1)

    for i in range(NITER):
        step(i)

    # drain the last delayed iteration
    k = NITER - 1
    if k % 4 == 0:
        phi_quads[k // 4] = emit_quad_seed(k // 4)
    emit_h2_phi(k, s2_live.pop(k), phi_quads[k // 4])
    g = k // 4
    if REDSC:
        emit_stage(g, phi_quads[g])
    for j in range(4):
        emit_reduce(g, phi_quads[g], j)

    # --- final: psi = qmax - EPS*log(M) - b3 -----------------------------
    # With EPS=1e-7 the f32 logsumexp collapses to the row max: the exp of
    # the second-best gap underflows, so the correction is exactly -EPS*log(M)
    # (bounded by EPS*log(M) ~ 7.6e-7 in all cases - far below tolerance).
    fin = ctx.enter_context(tc.tile_pool(name="fin", bufs=1))
    base = fin.tile([128, 1], F32)
    # base = -b3 - EPS*log(M)
    nc.vector.tensor_scalar(
        out=base, in0=b3s, scalar1=-1.0, scalar2=-EPS * math.log(M),
        op0=ALU.mult, op1=ALU.add,
    )
    # combine the 4 chunk-maxima per quad, then add base
    qmaxq = fin.tile([128, 16], F32)
    qin = bass.AP(
        tensor=qmax4.tensor, offset=qmax4.offset,
        ap=[qmax4.ap[0], [4, 16], [1, 4]],
    )
    nc.vector.reduce_max(out=qmaxq, in_=qin, axis=mybir.AxisListType.X)
    qmaxc = fin.tile([128, 16], F32)
    nc.vector.tensor_scalar(
        out=qmaxc, in0=qmaxq, scalar1=base, scalar2=0.0,
        op0=ALU.add, op1=ALU.add,
    )
    # Row n = (4g+q) + 64p lives at qmaxc[32q+p, g]: one DMA per q writes
    # out[n] for (p, g) via a strided DRAM AP.
    for q in range(4):
        out_ap = bass.AP(
            tensor=out.tensor, offset=out.offset + q,
            ap=[[NFULL, 2], [4, 16]],
        )
        nc.sync.dma_start(out=out_ap, in_=qmaxc[32 * q : 32 * q + 2, :])


def kernel(**inputs):
    if "nc" not in _CACHE:
        _CACHE["nc"] = build_program()
    nc = _CACHE["nc"]

    f32 = lambda a: np.ascontiguousarray(np.asarray(a, dtype=np.float32))
    X = f32(inputs["X"])
    U = f32(inputs["U"])
    Y = f32(inputs["Y"])
    shared = dict(
        U=U,
        W1=f32(inputs["W1"]),
        b1=f32(inputs["b1"]),
        W2=f32(inputs["W2"]),
        b2=f32(inputs["b2"]),
        W3=f32(inputs["W3"]),
        b3=f32(inputs["b3"]),
    )
    in_maps = [
        dict(
            X=X[c * NLOC : (c + 1) * NLOC],
            Y=Y[c * NLOC : (c + 1) * NLOC],
            **shared,
        )
        for c in range(N_CORES)
    ]
    res = run_bass_kernel_spmd(nc, in_maps, core_ids=list(range(N_CORES)))
    return np.concatenate([res.results[c]["out"] for c in range(N_CORES)], axis=0)


if __name__ == "__main__":
    rng = np.random.default_rng(0)
    ins = {
        "X": rng.standard_normal((N, DX), dtype=np.float32),
        "U": rng.standard_normal((M, DY), dtype=np.float32),
        "Y": rng.standard_normal((N, DY), dtype=np.float32),
        "W1": (rng.standard_normal((DX + DY, H)) * 0.1).astype(np.float32),
        "b1": np.zeros(H, np.float32),
        "W2": (rng.standard_normal((H, H)) * 0.1).astype(np.float32),
        "b2": np.zeros(H, np.float32),
        "W3": (rng.standard_normal((H, 1)) * 0.1).astype(np.float32),
        "b3": np.zeros(1, np.float32),
    }
    out = kernel(**ins)
    print(out.shape, out[:4, 0])
